# revision 1
# baseline (speedup 1.0000x reference)
"""AHGNN_LP distributed Trainium2 kernel (8 NeuronCores).

Full graph pipeline computed on device, row-sharded (256 rows/core):
  - connectivity fitness Z@Z.T  (bf16 hi/lo 3-matmul split, f32-grade accuracy)
  - structural fitness: dense segment-softmax over the edge-count matrix C
    (C built host-side from edge_index: pure data-layout densification,
     handles duplicate edges exactly)
  - A1 / A2 = 2-hop closure via fp8 0/1 matmul (exact), cluster scores,
    local-extrema selection, assignment-matrix masking
  - pooled = S_f.T @ emb via AllToAll column redistribution + bf16 matmul
Collectives: 1 packed AllGather (Z.T hi/lo + s2), AllGather(A1), ReduceScatter
(cluster col-sums), 3 tiny AllGathers (scores / is_cluster / reduced), AllToAll.
"""

import os
import sys

import numpy as np
import ml_dtypes

sys.path.insert(0, "/opt/trn_rl_repo")

N = 2048
D = 512
NC = 8
P = N // NC          # 256 rows per core
T = P // 128         # 2 partition tiles per core
K4 = D // 128        # 4 K-tiles over D
KN = N // 128        # 16 K-tiles over N
CH = N // 512        # 4 free-dim chunks of 512

BF16 = ml_dtypes.bfloat16
FP8 = ml_dtypes.float8_e4m3

_CACHE = {}


def _build():
    import concourse.bacc as bacc
    import concourse.mybir as mybir
    from concourse import tile

    dt = mybir.dt
    ALU = mybir.AluOpType
    AF = mybir.ActivationFunctionType
    AX = mybir.AxisListType

    nc = bacc.Bacc("TRN2", target_bir_lowering=False, debug=False, num_devices=NC)

    d_embp = nc.dram_tensor("emb_p", [P, D], dt.float32, kind="ExternalInput")
    d_embf = nc.dram_tensor("emb_bf", [N, D], dt.bfloat16, kind="ExternalInput")
    d_cnt = nc.dram_tensor("c_cnt", [P, N], dt.bfloat16, kind="ExternalInput")
    d_mat = nc.dram_tensor("mat", [P, N], dt.bfloat16, kind="ExternalInput")
    d_id = nc.dram_tensor("ident", [P, N], dt.bfloat16, kind="ExternalInput")
    d_w12r = nc.dram_tensor("w12r", [2, D], dt.float32, kind="ExternalInput")
    d_bsc = nc.dram_tensor("bsc", [1, 1], dt.float32, kind="ExternalInput")
    d_idf = nc.dram_tensor("id128f", [128, 128], dt.float32, kind="ExternalInput")
    d_ide = nc.dram_tensor("id128e", [128, 128], dt.bfloat16, kind="ExternalInput")
    d_out = nc.dram_tensor("out", [P, D], dt.float32, kind="ExternalOutput")
    d_dbg = nc.dram_tensor("dbg", [P, 8], dt.float32, kind="ExternalOutput")

    RG = [list(range(NC))]

    with tile.TileContext(nc) as tc:
        from contextlib import ExitStack

        es = ExitStack()
        Pc = es.enter_context(tc.tile_pool(name="const", bufs=1))
        Pp = es.enter_context(tc.tile_pool(name="persist", bufs=1))
        Pv = es.enter_context(tc.tile_pool(name="vchain", bufs=3))
        Ps = es.enter_context(tc.tile_pool(name="scr", bufs=1))
        Pt = es.enter_context(tc.tile_pool(name="tiny", bufs=1))
        Qmm = es.enter_context(tc.tile_pool(name="psmm", bufs=4, space="PSUM"))
        Qtr = es.enter_context(tc.tile_pool(name="pstr", bufs=2, space="PSUM"))
        Qtn = es.enter_context(tc.tile_pool(name="pstn", bufs=2, space="PSUM"))
        Pd = es.enter_context(tc.tile_pool(name="dram", bufs=1, space="DRAM"))

        def tl(pool, shape, dty, name, tag=None, bufs=None):
            kw = {} if bufs is None else {"bufs": bufs}
            return pool.tile(shape, dty, name=name, tag=tag or name, **kw)

        def bcast_rows(dst, dram_ap):
            """DMA-broadcast a DRAM vector into all 128 partitions."""
            flat = dram_ap.rearrange("a b -> (a b)")
            nc.sync.dma_start(dst, flat.unsqueeze(0).broadcast_to(
                [128, flat.shape[0]]))

        # ---------------- constants ----------------
        idf = tl(Pc, [128, 128], dt.float32, "idf")
        ide = tl(Pc, [128, 128], dt.bfloat16, "ide")
        nc.sync.dma_start(idf[:], d_idf[:])
        nc.sync.dma_start(ide[:], d_ide[:])
        bvec = tl(Pc, [128, 1], dt.float32, "bvec")
        onesb = tl(Pc, [128, 1], dt.bfloat16, "onesb")
        nc.vector.memset(onesb[:], 1.0)

        bcast_rows(bvec[:], d_bsc[:])
        Isball = tl(Pp, [128, T * N], dt.bfloat16, "Isball")
        nc.sync.dma_start(Isball[:],
                          d_id[:].rearrange("(t p) n -> p t n", p=128))
        Isb = [Isball[:, N * t:N * (t + 1)] for t in range(T)]

        def tv(name, dty=None):
            return [tl(Pt, [128, 1], dty or dt.float32, f"{name}{t}")
                    for t in range(T)]

        ss, nrm, rinv, s1c, s2c = tv("ss"), tv("nrm"), tv("rinv"), tv("s1c"), tv("s2c")
        mneg, den, dinv = tv("mneg"), tv("den"), tv("dinv")
        deg1, num1, deg2c, num2p, colsum, sco = (tv("deg1"), tv("num1"),
                                                 tv("deg2c"), tv("num2p"),
                                                 tv("colsum"), tv("sco"))
        lemax, reds, notred, m1 = tv("lemax"), tv("reds"), tv("notred"), tv("m1")
        iscl_f = tv("iscl_f")
        iscl = tv("iscl", dt.bfloat16)

        struct_ = [tl(Pp, [128, N], dt.float32, f"struct{t}") for t in range(T)]
        fitness = [tl(Pp, [128, N], dt.float32, f"fitness{t}") for t in range(T)]
        s2b = tl(Pp, [128, N], dt.float32, "s2b", tag="bcf32")

        with tc.tile_pool(name="earlyA", bufs=1) as Pe, \
             tc.tile_pool(name="ztpool", bufs=1) as Pzt, \
             tc.tile_pool(name="csbpool", bufs=1) as Pcs:
            embpall = tl(Pe, [128, T * D], dt.float32, "embpall")
            Csball = tl(Pcs, [128, T * N], dt.bfloat16, "Csball")
            nc.sync.dma_start(embpall[:],
                              d_embp[:].rearrange("(t p) d -> p t d", p=128))
            nc.sync.dma_start(Csball[:],
                              d_cnt[:].rearrange("(t p) n -> p t n", p=128))
            embp = [embpall[:, D * t:D * (t + 1)] for t in range(T)]
            Csb = [Csball[:, N * t:N * (t + 1)] for t in range(T)]

            # ---- s1/s2 row-dots first, so AG#0 fires immediately ----
            w1b = tl(Pe, [128, D], dt.float32, "w1b")
            w2b = tl(Pe, [128, D], dt.float32, "w2b")
            bcast_rows(w1b[:], d_w12r[0:1, :])
            bcast_rows(w2b[:], d_w12r[1:2, :])
            for t in range(T):
                sd = tl(Ps, [128, D], dt.float32, f"sd{t}", tag="scrf32", bufs=2)
                nc.vector.tensor_tensor(sd[:], embp[t], w1b[:], ALU.mult)
                nc.vector.tensor_reduce(s1c[t][:], sd[:], AX.X, ALU.add)
                nc.vector.tensor_scalar(s1c[t][:], s1c[t][:], bvec[:], None,
                                        ALU.add)
                sd2 = tl(Ps, [128, D], dt.float32, f"sd2{t}", tag="scrf32", bufs=2)
                nc.vector.tensor_tensor(sd2[:], embp[t], w2b[:], ALU.mult)
                nc.vector.tensor_reduce(s2c[t][:], sd2[:], AX.X, ALU.add)
            ag0i = Pd.tile([T, 128], dt.float32, name="ag0i")
            ag0o = Pd.tile([T * NC, 128], dt.float32, name="ag0o",
                           addr_space="Shared")
            for t in range(T):
                nc.sync.dma_start(ag0i[t:t + 1, :], s2c[t][:])
            nc.gpsimd.collective_compute("AllGather", ALU.bypass,
                                         replica_groups=RG,
                                         ins=[ag0i[:]], outs=[ag0o[:]])
            bcast_rows(s2b[:], ag0o[:])

            # ---- normalize -> Z -> Z^T (+ bf16 hi/lo split) ----
            Zt = [tl(Pv, [128, D], dt.float32, f"Zt{t}", tag="vchain")
                  for t in range(T)]
            ZpTall = tl(Pzt, [128, K4 * P], dt.float32, "ZpTall")
            ZpT = [ZpTall[:, P * k:P * (k + 1)] for k in range(K4)]
            ZhTall = tl(Pzt, [128, K4 * P], dt.bfloat16, "ZhTall")
            ZlTall = tl(Pzt, [128, K4 * P], dt.bfloat16, "ZlTall")
            for t in range(T):
                sq = tl(Ps, [128, D], dt.float32, f"sq{t}", tag="scrf32", bufs=2)
                nc.scalar.activation(sq[:], embp[t], AF.Square,
                                     accum_out=ss[t][:])
                nc.scalar.sqrt(nrm[t][:], ss[t][:])
                nc.vector.tensor_scalar(nrm[t][:], nrm[t][:], 1e-12, None,
                                        ALU.max)
                nc.vector.reciprocal(rinv[t][:], nrm[t][:])
                nc.vector.tensor_scalar(Zt[t][:], embp[t], rinv[t][:], None,
                                        ALU.mult)
            for t in range(T):
                for k in range(K4):
                    pt = Qtr.tile([128, 128], dt.float32, name=f"ptr{t}{k}",
                                  tag="ptr")
                    nc.tensor.transpose(pt[:], Zt[t][:, 128 * k:128 * (k + 1)],
                                        idf[:])
                    nc.scalar.copy(ZpT[k][:, 128 * t:128 * (t + 1)], pt[:])

            nc.vector.tensor_copy(ZhTall[:], ZpTall[:])
            nc.vector.tensor_tensor(ZlTall[:], ZpTall[:], ZhTall[:],
                                    ALU.subtract)
            # ---- AG#1h / AG#1l: Z^T hi then lo (bf16) ----
            ag1hi = Pd.tile([K4 * 128, P], dt.bfloat16, name="ag1hi")
            ag1ho = Pd.tile([K4 * 128 * NC, P], dt.bfloat16, name="ag1ho",
                            addr_space="Shared")
            ag1li = Pd.tile([K4 * 128, P], dt.bfloat16, name="ag1li")
            ag1lo = Pd.tile([K4 * 128 * NC, P], dt.bfloat16, name="ag1lo",
                            addr_space="Shared")
            nc.sync.dma_start(
                ag1hi[:].rearrange("(k p) n -> p k n", p=128), ZhTall[:])
            nc.gpsimd.collective_compute("AllGather", ALU.bypass,
                                         replica_groups=RG,
                                         ins=[ag1hi[:]], outs=[ag1ho[:]])
            nc.sync.dma_start(
                ag1li[:].rearrange("(k p) n -> p k n", p=128), ZlTall[:])
            nc.gpsimd.collective_compute("AllGather", ALU.bypass,
                                         replica_groups=RG,
                                         ins=[ag1li[:]], outs=[ag1lo[:]])
            ZTh = [tl(Pzt, [128, N], dt.bfloat16, f"ZTh{k}") for k in range(K4)]
            ZTl = [tl(Pzt, [128, N], dt.bfloat16, f"ZTl{k}") for k in range(K4)]
            ag1hv = ag1ho[:].rearrange("(r k p) n -> k p r n", r=NC, k=K4)
            ag1lv = ag1lo[:].rearrange("(r k p) n -> k p r n", r=NC, k=K4)
            for k in range(K4):
                nc.sync.dma_start(ZTh[k][:], ag1hv[k])
            for k in range(K4):
                nc.sync.dma_start(ZTl[k][:], ag1lv[k])

            # ---- structural softmax (dense, f32) ----
            for t in range(T):
                V = tl(Pv, [128, N], dt.float32, f"V{t}", tag="vchain")
                t001 = tl(Pv, [128, N], dt.float32, f"t001{t}", tag="vchain")
                Vlr = tl(Pv, [128, N], dt.float32, f"Vlr{t}", tag="vchain")
                E = tl(Pv, [128, N], dt.float32, f"E{t}", tag="vchain")
                U = tl(Pv, [128, N], dt.float32, f"U{t}", tag="vchain")
                nc.vector.tensor_scalar(V[:], s2b[:], s1c[t][:], None, ALU.add)
                nc.vector.tensor_scalar(t001[:], s2b[:], s1c[t][:], 0.01,
                                        ALU.add, ALU.mult)
                nc.vector.tensor_tensor(Vlr[:], V[:], t001[:], ALU.max)
                nc.vector.tensor_reduce(mneg[t][:], Vlr[:], AX.X, ALU.max,
                                        negate=True)
                nc.scalar.activation(E[:], Vlr[:], AF.Exp, bias=mneg[t][:])
                nc.vector.tensor_tensor(U[:], E[:], Csb[t], ALU.mult)
                dsc = tl(Ps, [128, N], dt.float32, f"dsc{t}", tag="scrf32",
                         bufs=2)
                nc.scalar.activation(dsc[:], U[:], AF.Copy,
                                     accum_out=den[t][:])
                nc.vector.tensor_scalar(den[t][:], den[t][:], 1e-16, None,
                                        ALU.add)
                nc.vector.reciprocal(dinv[t][:], den[t][:])
                nc.scalar.activation(struct_[t][:], U[:], AF.Copy,
                                     scale=dinv[t][:])

            # ---- conn = hh + hl + lh (bf16 hi/lo, f32-grade) + fitness ----
            for t in range(T):
                for c in range(CH):
                    ps = Qmm.tile([128, 512], dt.float32, name=f"conn{t}{c}",
                                  tag="mm")
                    first = True
                    for (Lh, Rh) in ((ZhTall, ZTh), (ZhTall, ZTl),
                                     (ZlTall, ZTh)):
                        for k in range(K4):
                            nc.tensor.matmul(
                                ps[:],
                                Lh[:, P * k + 128 * t:P * k + 128 * (t + 1)],
                                Rh[k][:, 512 * c:512 * (c + 1)],
                                start=first,
                                stop=(Lh is ZlTall and k == K4 - 1))
                            first = False
                    nc.vector.tensor_tensor(
                        fitness[t][:, 512 * c:512 * (c + 1)], ps[:],
                        struct_[t][:, 512 * c:512 * (c + 1)], ALU.add)

        # ---- A1, deg1, num1 ----
        A1b = [tl(Pp, [128, N], dt.bfloat16, f"A1b{t}") for t in range(T)]
        with tc.tile_pool(name="matpool", bufs=1) as Pm:
            matall = tl(Pm, [128, T * N], dt.bfloat16, "matall")
            nc.sync.dma_start(matall[:],
                              d_mat[:].rearrange("(t p) n -> p t n", p=128))
            matsb = [matall[:, N * t:N * (t + 1)] for t in range(T)]
            for t in range(T):
                fm = tl(Ps, [128, N], dt.float32, f"fm{t}", tag="scrf32",
                        bufs=2)
                nc.vector.tensor_tensor(fm[:], fitness[t][:], matsb[t],
                                        ALU.mult)
                nc.vector.tensor_scalar(A1b[t][:], fm[:], 0.0, None, ALU.is_gt,
                                        ALU.add, accum_out=deg1[t][:])
                rl = tl(Ps, [128, N], dt.float32, f"rl{t}", tag="scrf32",
                        bufs=2)
                nc.scalar.activation(rl[:], fm[:], AF.Relu,
                                     accum_out=num1[t][:])

        Pmid_cm = tc.tile_pool(name="mid", bufs=1)
        Pmid = Pmid_cm.__enter__()
        embfall = tl(Pmid, [128, KN * D], dt.bfloat16, "embfall")
        nc.sync.dma_start(embfall[:],
                          d_embf[:].rearrange("(k p) d -> p k d", p=128))
        A1Tall = tl(Pp, [128, KN * P], dt.float8e4, "A1Tall")
        A1T = [A1Tall[:, P * k:P * (k + 1)] for k in range(KN)]

        # ---- AG#2: A1 (fp8) + local transpose ----
        ag2i = Pd.tile([P, N], dt.float8e4, name="ag2i")
        ag2o = Pd.tile([N, N], dt.float8e4, name="ag2o", addr_space="Shared")
        for t in range(T):
            a18 = tl(Ps, [128, N], dt.float8e4, f"a18_{t}", tag="scrbf", bufs=3)
            nc.vector.tensor_copy(a18[:], A1b[t][:])
            nc.sync.dma_start(ag2i[128 * t:128 * (t + 1), :], a18[:])
        nc.gpsimd.collective_compute("AllGather", ALU.bypass, replica_groups=RG,
                                     ins=[ag2i[:]], outs=[ag2o[:]])
        for t in range(T):
            for k in range(KN):
                pt = Qtr.tile([128, 128], dt.bfloat16, name=f"pa{t}{k}",
                              tag="ptr")
                nc.tensor.transpose(pt[:], A1b[t][:, 128 * k:128 * (k + 1)],
                                    ide[:])
                nc.scalar.copy(A1T[k][:, 128 * t:128 * (t + 1)], pt[:])

        cluster = [tl(Pp, [128, N], dt.bfloat16, f"cluster{t}") for t in range(T)]
        with tc.tile_pool(name="a1fpool", bufs=1) as Pa1f:
            A1Fall = tl(Pa1f, [128, KN * N], dt.float8e4, "A1Fall")
            ag2v = ag2o[:].rearrange("(g p) n -> p g n", p=128)
            for g in range(4):
                nc.sync.dma_start(
                    A1Fall[:, 4 * N * g:4 * N * (g + 1)],
                    ag2o[:].rearrange("(g k p) n -> g p k n", g=4, k=4)[g])

            # ---- cnt = A1@A1 (fp8), A2, cluster ----
            for t in range(T):
                g = tl(Ps, [128, N], dt.bfloat16, f"g{t}", tag="scrbf", bufs=3)
                for c in range(CH):
                    ps = Qmm.tile([128, 512], dt.float32, name=f"cnt{t}{c}",
                                  tag="mm")
                    for gg in range(KN // 2):
                        lhs3 = A1Tall[:, P * 2 * gg + 128 * t:
                                      P * (2 * gg + 1) + 128 * (t + 1)]
                        lhs3 = A1Tall[:].rearrange(
                            "p (k m) -> p k m", k=KN)[
                            :, 2 * gg:2 * gg + 2,
                            128 * t:128 * (t + 1)]
                        rhs3 = A1Fall[:].rearrange(
                            "p (k n) -> p k n", k=KN)[
                            :, 2 * gg:2 * gg + 2,
                            512 * c:512 * (c + 1)]
                        nc.tensor.matmul(
                            ps[:], lhs3, rhs3,
                            perf_mode=mybir.MatmulPerfMode.DoubleRow,
                            start=(gg == 0), stop=(gg == KN // 2 - 1))
                    nc.vector.tensor_scalar(g[:, 512 * c:512 * (c + 1)], ps[:],
                                            0.5, None, ALU.is_ge)
                dd = tl(Ps, [128, N], dt.bfloat16, f"dd{t}", tag="scrbf",
                        bufs=3)
                a2t = tl(Ps, [128, N], dt.bfloat16, f"a2t{t}", tag="scrbf",
                         bufs=3)
                nc.vector.tensor_tensor(dd[:], g[:], A1b[t][:], ALU.subtract)
                nc.vector.tensor_tensor(dd[:], dd[:], Isb[t], ALU.subtract)
                nc.vector.tensor_scalar(a2t[:], dd[:], 0.0, None, ALU.max,
                                        ALU.add, accum_out=deg2c[t][:])
                nc.vector.tensor_tensor(cluster[t][:], A1b[t][:], a2t[:],
                                        ALU.add)

        Pmid2_cm = tc.tile_pool(name="mid2", bufs=1)
        Pmid2 = Pmid2_cm.__enter__()
        embp2 = tl(Pmid2, [128, T * D], dt.float32, "embp2")
        nc.sync.dma_start(embp2[:],
                          d_embp[:].rearrange("(t p) d -> p t d", p=128))
        # ---- cluster col-sums -> ReduceScatter ----
        rsi = Pd.tile([1, N], dt.float32, name="rsi")
        rso = Pd.tile([1, P], dt.float32, name="rso")
        cssall = tl(Ps, [1, N], dt.float32, "cssall", tag="csst", bufs=1)
        for c in range(CH):
            pcs = Qtn.tile([1, 512], dt.float32, name=f"pcs{c}", tag="pcs")
            for t in range(T):
                nc.tensor.matmul(pcs[:], onesb[:],
                                 cluster[t][:, 512 * c:512 * (c + 1)],
                                 start=(t == 0), stop=(t == T - 1))
            nc.scalar.copy(cssall[0:1, 512 * c:512 * (c + 1)], pcs[:])
        nc.sync.dma_start(rsi[:], cssall[:])
        nc.gpsimd.collective_compute("ReduceScatter", ALU.add, replica_groups=RG,
                                     ins=[rsi[:]], outs=[rso[:]])
        for t in range(T):
            nc.sync.dma_start(colsum[t][:], rso[0:1, 128 * t:128 * (t + 1)])

        # ---- S_f_pre, num2', scores; A2A payload fires as soon as ready ----
        Sfpre = [tl(Pp, [128, N], dt.bfloat16, f"Sfpre{t}", tag="sfx", bufs=2)
                 for t in range(T)]
        a2i = Pd.tile([N, P], dt.bfloat16, name="a2i")
        a2o = Pd.tile([N, P], dt.bfloat16, name="a2o")
        a2iv = a2i[:].rearrange("(q t p) n -> t p q n", q=NC, t=T, p=128)
        for t in range(T):
            sfs = tl(Ps, [128, N], dt.float32, f"sfs{t}", tag="scrf32", bufs=2)
            nc.vector.tensor_tensor(sfs[:], cluster[t][:], fitness[t][:],
                                    ALU.mult)
            nc.scalar.activation(Sfpre[t][:], sfs[:], AF.Copy,
                                 accum_out=num2p[t][:])
            nc.sync.dma_start(a2iv[t], Sfpre[t][:])
        nc.gpsimd.collective_compute("AllToAll", ALU.bypass, replica_groups=RG,
                                     ins=[a2i[:]], outs=[a2o[:]])
        Yall = tl(Pmid2, [128, KN * P], dt.bfloat16, "Yall")
        nc.sync.dma_start(Yall[:],
                          a2o[:].rearrange("(k p) n -> p k n", p=128))

        for t in range(T):
            n2 = tl(Pt, [128, 1], dt.float32, f"n2_{t}")
            d2 = tl(Pt, [128, 1], dt.float32, f"d2_{t}")
            r1 = tl(Pt, [128, 1], dt.float32, f"r1_{t}")
            r2 = tl(Pt, [128, 1], dt.float32, f"r2_{t}")
            q1 = tl(Pt, [128, 1], dt.float32, f"q1_{t}")
            nc.vector.tensor_tensor(n2[:], num2p[t][:], num1[t][:],
                                    ALU.subtract)
            nc.vector.tensor_copy(d2[:], deg2c[t][:])
            nc.vector.tensor_scalar(r1[:], deg1[t][:], 1.0, None, ALU.max)
            nc.vector.reciprocal(r1[:], r1[:])
            nc.vector.tensor_scalar(d2[:], d2[:], 1.0, None, ALU.max)
            nc.vector.reciprocal(r2[:], d2[:])
            nc.vector.tensor_tensor(q1[:], num1[t][:], r1[:], ALU.mult)
            nc.vector.tensor_tensor(r2[:], n2[:], r2[:], ALU.mult)
            nc.vector.tensor_tensor(q1[:], q1[:], r2[:], ALU.add)
            nc.vector.tensor_scalar(sco[t][:], q1[:], 0.5, None, ALU.mult)

        # ---- AG#3 scores -> lemax -> is_cluster ----
        ag3i = Pd.tile([T, 128], dt.float32, name="ag3i")
        ag3o = Pd.tile([T * NC, 128], dt.float32, name="ag3o",
                       addr_space="Shared")
        for t in range(T):
            nc.sync.dma_start(ag3i[t:t + 1, :], sco[t][:])
        nc.gpsimd.collective_compute("AllGather", ALU.bypass, replica_groups=RG,
                                     ins=[ag3i[:]], outs=[ag3o[:]])
        scb = tl(Pp, [128, N], dt.float32, "scb", tag="bcf32")
        bcast_rows(scb[:], ag3o[:])
        for t in range(T):
            le = tl(Ps, [128, N], dt.float32, f"le{t}", tag="scrf32", bufs=2)
            nc.vector.tensor_tensor(le[:], A1b[t][:], scb[:], ALU.mult)
            nc.vector.tensor_reduce(lemax[t][:], le[:], AX.X, ALU.max)
            nc.vector.tensor_tensor(iscl_f[t][:], sco[t][:], lemax[t][:],
                                    ALU.is_gt)
            nc.vector.tensor_copy(iscl[t][:], iscl_f[t][:])

        # ---- AG#4 is_cluster -> reduced -> row masks ----
        ag4i = Pd.tile([T, 128], dt.bfloat16, name="ag4i")
        ag4o = Pd.tile([T * NC, 128], dt.bfloat16, name="ag4o",
                       addr_space="Shared")
        for t in range(T):
            nc.sync.dma_start(ag4i[t:t + 1, :], iscl[t][:])
        nc.gpsimd.collective_compute("AllGather", ALU.bypass, replica_groups=RG,
                                     ins=[ag4i[:]], outs=[ag4o[:]])
        isb = tl(Pp, [128, N], dt.bfloat16, "isb")
        bcast_rows(isb[:], ag4o[:])
        for t in range(T):
            rs_ = tl(Ps, [128, N], dt.bfloat16, f"rs_{t}", tag="scrbf", bufs=3)
            rs2 = tl(Ps, [128, N], dt.bfloat16, f"rs2{t}", tag="scrbf", bufs=3)
            nc.vector.tensor_tensor(rs_[:], cluster[t][:], isb[:], ALU.mult)
            nc.scalar.activation(rs2[:], rs_[:], AF.Copy, accum_out=reds[t][:])
            ra = tl(Pt, [128, 1], dt.float32, f"ra{t}")
            rb = tl(Pt, [128, 1], dt.float32, f"rb{t}")
            nc.vector.tensor_scalar(ra[:], reds[t][:], 0.0, None, ALU.is_gt)
            nc.vector.tensor_scalar(rb[:], colsum[t][:], 0.0, None,
                                    ALU.is_equal)
            nc.vector.tensor_tensor(ra[:], ra[:], rb[:], ALU.max)
            nc.vector.tensor_scalar(notred[t][:], ra[:], -1.0, 1.0, ALU.mult,
                                    ALU.add)
            nc.vector.tensor_tensor(m1[t][:], iscl_f[t][:], notred[t][:],
                                    ALU.mult)

        for t in range(T):
            dbgt = tl(Pp, [128, 8], dt.float32, f"dbgt{t}")
            for j, vec in enumerate([deg1, num1, deg2c, num2p, sco, lemax, reds,
                                     colsum]):
                nc.vector.tensor_copy(dbgt[:, j:j + 1], vec[t][:])
            nc.sync.dma_start(d_dbg[128 * t:128 * (t + 1), :], dbgt[:])

        # ---- pooled matmul with receiver-side masks ----
        for t in range(T):
            pp = Qmm.tile([128, 512], dt.float32, name=f"pool{t}", tag="mm")
            for k in range(KN):
                nc.tensor.matmul(
                    pp[:], Yall[:, P * k + 128 * t:P * k + 128 * (t + 1)],
                    embfall[:, D * k:D * (k + 1)],
                    start=(k == 0), stop=(k == KN - 1))
            osb = tl(Pmid2, [128, 512], dt.float32, f"osb{t}", tag="osbt",
                     bufs=2)
            emt = tl(Pmid2, [128, 512], dt.float32, f"emt{t}", tag="emtt",
                     bufs=2)
            nc.vector.tensor_scalar(osb[:], pp[:], m1[t][:], None, ALU.mult)
            nc.vector.tensor_scalar(emt[:], embp2[:, D * t:D * (t + 1)],
                                    notred[t][:], None, ALU.mult)
            nc.vector.tensor_tensor(osb[:], osb[:], emt[:], ALU.add)
            nc.sync.dma_start(d_out[128 * t:128 * (t + 1), :], osb[:])

        Pmid2_cm.__exit__(None, None, None)
        Pmid_cm.__exit__(None, None, None)
        es.close()

    nc.compile()
    return nc


def _prep_inputs(embedding, edge_index, W_score, b_score):
    emb = np.ascontiguousarray(embedding, dtype=np.float32)
    ei = np.asarray(edge_index)
    src = ei[0].astype(np.int64)
    dst = ei[1].astype(np.int64)
    C = np.zeros((N, N), np.float32)
    np.add.at(C, (src, dst), 1.0)
    mat = (C > 0).astype(np.float32)
    np.fill_diagonal(mat, 0.0)
    ident = np.eye(N, dtype=np.float32)
    emb_bf = emb.astype(BF16)
    C_bf = C.astype(BF16)
    mat_bf = mat.astype(BF16)
    id_bf = ident.astype(BF16)
    w12r = np.ascontiguousarray(
        np.stack([W_score[:D, 0], W_score[D:, 0]], axis=0), dtype=np.float32)
    bsc = np.asarray(b_score, dtype=np.float32).reshape(1, 1)
    id128f = np.eye(128, dtype=np.float32)
    id128e = np.eye(128, dtype=np.float32).astype(BF16)
    in_maps = []
    for p in range(NC):
        rows = slice(P * p, P * (p + 1))
        in_maps.append({
            "emb_p": emb[rows],
            "emb_bf": emb_bf,
            "c_cnt": np.ascontiguousarray(C_bf[rows]),
            "mat": np.ascontiguousarray(mat_bf[rows]),
            "ident": np.ascontiguousarray(id_bf[rows]),
            "w12r": w12r,
            "bsc": bsc,
            "id128f": id128f,
            "id128e": id128e,
        })
    return in_maps


def kernel(embedding, edge_index, edge_matrix, edge_matrix_weight, W_score, b_score,
           _trace=False):
    if "nc" not in _CACHE:
        _CACHE["nc"] = _build()
    nc = _CACHE["nc"]
    from concourse import bass_utils
    in_maps = _prep_inputs(embedding, edge_index, W_score, b_score)
    res = bass_utils.run_bass_kernel_spmd(nc, in_maps, core_ids=list(range(NC)),
                                          trace=_trace)
    _CACHE["last_results"] = res
    out = np.concatenate([res.results[p]["out"] for p in range(NC)], axis=0)
    return out.astype(np.float32)

